# revision 9
# baseline (speedup 1.0000x reference)
"""MultiHeadAttention Trainium2 kernel (B=4, S=2048, D=1024, H=16, Dh=64).

Sharding: 8 cores = 4 batches x 2 head-groups (8 heads each).  Each core
computes QKV projections for its (batch, head-group), full attention for its
8 heads, and a partial output projection (row-parallel over Wo).  The host
sums the two per-batch partials and adds the output bias.

Schedule (all matmuls bf16 inputs, fp32 PSUM accumulation):
  - K-proj and V-proj run first (streamed kT/vT over two DMA queues), in
    4-bank PSUM groups so drains overlap the next group's matmuls.
  - Attention runs qb-outer (4 query blocks x 8 heads).  The exp stream on
    ScalarE is the window bottleneck (~277us); Q-proj for qb+1 and the
    output projection for qb-1 slot into the PE slack between heads.
  - ctxT is stored head-PAIR packed [128, 4, S] (head 2p on partitions 0:64,
    head 2p+1 on 64:128) so out-proj matmuls contract a full K=128 of real
    data (no zero padding).  Odd heads' normalized ctx is staged through a
    [64,512] tile and DMA-moved to partitions 64:128.
  - scores matmuls keep the zero-padded K=128 contraction: half-array (K=64)
    matmuls never satisfy the PE activity monitor, pinning the clock at
    1.2 GHz.  Padding is free (matmul time is set by the moving free dim).
  - V is stored with a ones-column per head (V_ext [s, h, 65]) so the
    attention-value matmul also produces the softmax denominator (row 64 of
    ps_ctx).  Denominator -> reciprocal_approx_fast -> partition broadcast
    -> multiply.
"""

import sys

for _p in ("/opt/trn_rl_repo", "/root/.axon_site/_ro/trn_rl_repo"):
    if _p not in sys.path:
        sys.path.append(_p)

import numpy as np
import ml_dtypes

import concourse.bass as bass
import concourse.tile as tile
from concourse import bacc, mybir
from concourse.bass_utils import run_bass_kernel_spmd

BF16 = ml_dtypes.bfloat16
F32 = mybir.dt.float32
BF = mybir.dt.bfloat16

D_MODEL = 1024
NUM_HEADS = 16
HEAD_DIM = 64
B, S = 4, 2048
HPC = 8          # heads per core
DHG = HPC * HEAD_DIM  # 512, head dims per core
NPAIR = HPC // 2      # 4 head pairs per core

# knobs read by test.py
TRACE = False
TRACE_CORES = None
LAST_RESULT = None

_PROGRAM_CACHE = {}


def _build_program(kt: int) -> bass.Bass:
    """Build the per-core SPMD program. kt = contraction tiles over d_model
    (8 normally, 9 when inputs are augmented with a bias row)."""
    nc = bacc.Bacc("TRN2", debug=False, target_bir_lowering=False)

    kd = kt * 128
    qT = nc.dram_tensor("qT", [kd, S], BF, kind="ExternalInput").ap()
    kT = nc.dram_tensor("kT", [kd, S], BF, kind="ExternalInput").ap()
    vT = nc.dram_tensor("vT", [kd, S], BF, kind="ExternalInput").ap()
    wqT = nc.dram_tensor("wqT", [kd, DHG], BF, kind="ExternalInput").ap()
    wkT = nc.dram_tensor("wkT", [kd, DHG], BF, kind="ExternalInput").ap()
    wvT = nc.dram_tensor("wvT", [kd, DHG], BF, kind="ExternalInput").ap()
    woTp = nc.dram_tensor("woTp", [128, NPAIR, D_MODEL], BF,
                          kind="ExternalInput").ap()
    out = nc.dram_tensor("out", [S, D_MODEL], F32, kind="ExternalOutput").ap()

    with tile.TileContext(nc) as tc:
        _body(tc, qT, kT, vT, wqT, wkT, wvT, woTp, out, kt)
    nc.compile()
    return nc


def _body(tc, qT, kT, vT, wqT, wkT, wvT, woTp, out, kt):
    nc = tc.nc
    EXP = mybir.ActivationFunctionType.Exp

    with (
        tc.tile_pool(name="consts", bufs=1) as consts,
        tc.tile_pool(name="big", bufs=1) as big,
    ):
        sb_wq = consts.tile([128, kt, DHG], BF)
        sb_wo = consts.tile([128, NPAIR, D_MODEL], BF)
        sb_qres = consts.tile([128, kt, S], BF)
        sb_QT = big.tile([128, HPC, S], BF)
        sb_KT = big.tile([128, HPC, S], BF)
        sb_V = big.tile([128, 16, HPC, 65], BF)  # [s%128, s//128, h, :]
        sb_ctxT = big.tile([128, NPAIR, S], BF)

        # Scores lhsT/rhs dead partitions must be ZERO (not garbage: random
        # SBUF bits can decode to NaN and 0*NaN = NaN).  KT first: its zeros
        # gate the K-proj copies at ~10us.
        nc.vector.memset(sb_KT, 0.0)
        nc.vector.memset(sb_V[:, :, :, 64:65], 1.0)
        nc.vector.memset(sb_QT, 0.0)

        # ---- loads: sync queue carries wk + kT + wq + qres; the scalar
        # (ACT) HWDGE queue carries wv + vT + wo in parallel.  ACT is idle
        # during the projection phase so this doubles early DMA throughput.
        with tc.tile_pool(name="wkv", bufs=1) as wkv:
            sb_wk = wkv.tile([128, kt, DHG], BF)
            sb_wv = wkv.tile([128, kt, DHG], BF)
            nc.sync.dma_start(sb_wk, wkT.rearrange("(t p) m -> p t m", p=128))
            nc.scalar.dma_start(sb_wv,
                                wvT.rearrange("(t p) m -> p t m", p=128))

            with tc.tile_pool(name="ld", bufs=2 * kt) as loads:
                kin = []
                for nbp in range(2):
                    for ki in range(kt):
                        t = loads.tile([128, 1024], BF, tag="ld")
                        nc.sync.dma_start(
                            t, kT[ki * 128:(ki + 1) * 128,
                                  nbp * 1024:(nbp + 1) * 1024])
                        kin.append(t)
                nc.sync.dma_start(sb_wq,
                                  wqT.rearrange("(t p) m -> p t m", p=128))
                nc.sync.dma_start(sb_qres,
                                  qT.rearrange("(t p) f -> p t f", p=128))
                # vT tiles reuse the kT slots (pool round-robin); each load
                # waits for the K-proj reads of its slot to finish.
                vin = []
                for nbp in range(2):
                    for ki in range(kt):
                        t = loads.tile([128, 1024], BF, tag="ld")
                        nc.scalar.dma_start(
                            t, vT[ki * 128:(ki + 1) * 128,
                                  nbp * 1024:(nbp + 1) * 1024])
                        vin.append(t)
                nc.scalar.dma_start(sb_wo, woTp)

                # ---- K and V projections: 4-bank PSUM groups so each
                # group's drain copies overlap the next group's matmuls.
                with tc.tile_pool(name="ps_kv", bufs=8,
                                  space="PSUM") as psums:
                    for nbp in range(2):
                        for sg in range(2):     # s sub-blocks of 512
                            ps = [psums.tile([128, 512], F32, tag="pkv",
                                             name=f"pk{nbp}_{sg}_{i}")
                                  for i in range(NPAIR)]
                            for ki in range(kt):
                                t_in = kin[nbp * kt + ki]
                                for p in range(NPAIR):
                                    nc.tensor.matmul(
                                        ps[p],
                                        lhsT=sb_wk[:, ki,
                                                   p * 128:(p + 1) * 128],
                                        rhs=t_in[:, sg * 512:(sg + 1) * 512],
                                        start=(ki == 0), stop=(ki == kt - 1))
                            c0 = nbp * 1024 + sg * 512
                            for p in range(NPAIR):
                                for par in range(2):
                                    h = 2 * p + par
                                    sl = slice(par * 64, par * 64 + 64)
                                    nc.vector.tensor_copy(
                                        out=sb_KT[sl, h, c0:c0 + 512],
                                        in_=ps[p][sl, :])
                    # Q-proj for qb0 here: its matmuls become ready as soon
                    # as wq+qres land (~30us) and fill the PE stall while the
                    # first vT tiles wait on kT slot reuse.
                    for p in range(NPAIR):
                        psq = psums.tile([128, 512], F32, tag="pkv",
                                         name=f"pq0_{p}")
                        for ki in range(kt):
                            nc.tensor.matmul(
                                psq,
                                lhsT=sb_wq[:, ki, p * 128:(p + 1) * 128],
                                rhs=sb_qres[:, ki, 0:512],
                                start=(ki == 0), stop=(ki == kt - 1))
                        for par in range(2):
                            h = 2 * p + par
                            sl = slice(par * 64, par * 64 + 64)
                            nc.vector.tensor_copy(out=sb_QT[sl, h, 0:512],
                                                  in_=psq[sl, :])
                    # V: out[s-tile 128, dh 512]; lhsT = input tile.
                    for nbp in range(2):
                        for sg in range(2):     # st sub-groups of 4
                            ps = [psums.tile([128, 512], F32, tag="pkv",
                                             name=f"pv{nbp}_{sg}_{i}")
                                  for i in range(4)]
                            for ki in range(kt):
                                t_in = vin[nbp * kt + ki]
                                for i in range(4):
                                    sti = sg * 4 + i
                                    nc.tensor.matmul(
                                        ps[i],
                                        lhsT=t_in[:,
                                                  sti * 128:(sti + 1) * 128],
                                        rhs=sb_wv[:, ki, :],
                                        start=(ki == 0), stop=(ki == kt - 1))
                            for i in range(4):
                                st = nbp * 8 + sg * 4 + i
                                nc.vector.tensor_copy(
                                    out=sb_V[:, st, :, 0:64],
                                    in_=ps[i].rearrange("p (h d) -> p h d",
                                                        h=HPC))

        # ---- attention, qb-outer, with Q-proj/out-proj as PE filler -------
        with (
            tc.tile_pool(name="exps", bufs=4) as exps,
            tc.tile_pool(name="smalls", bufs=3) as smalls,
            tc.tile_pool(name="stage", bufs=2) as stage_pool,
            tc.tile_pool(name="outst", bufs=2) as outs_pool,
            tc.tile_pool(name="ps_scb", bufs=1, space="PSUM") as ps_big,
            tc.tile_pool(name="ps_scs", bufs=1, space="PSUM") as ps_small,
            tc.tile_pool(name="ps_cx", bufs=1, space="PSUM") as ps_cx_pool,
            tc.tile_pool(name="ps_fl", bufs=1, space="PSUM") as ps_fl_pool,
        ):
            def qproj_group(qb, p):
                q0 = qb * 512
                psq = ps_fl_pool.tile([128, 512], F32, tag="fl")
                for ki in range(kt):
                    nc.tensor.matmul(
                        psq,
                        lhsT=sb_wq[:, ki, p * 128:(p + 1) * 128],
                        rhs=sb_qres[:, ki, q0:q0 + 512],
                        start=(ki == 0), stop=(ki == kt - 1))
                for par in range(2):
                    h = 2 * p + par
                    sl = slice(par * 64, par * 64 + 64)
                    nc.vector.tensor_copy(out=sb_QT[sl, h, q0:q0 + 512],
                                          in_=psq[sl, :])

            def outproj_group(st):
                t_out = outs_pool.tile([128, D_MODEL], F32, tag="o")
                for nb2 in range(2):
                    ps_o = ps_fl_pool.tile([128, 512], F32, tag="fl")
                    for p in range(NPAIR):
                        nc.tensor.matmul(
                            ps_o,
                            lhsT=sb_ctxT[:, p, st * 128:(st + 1) * 128],
                            rhs=sb_wo[:, p, nb2 * 512:(nb2 + 1) * 512],
                            start=(p == 0), stop=(p == NPAIR - 1))
                    nc.vector.tensor_copy(
                        out=t_out[:, nb2 * 512:(nb2 + 1) * 512], in_=ps_o)
                nc.sync.dma_start(out[st * 128:(st + 1) * 128, :], t_out)

            # ktile groups per exp: alternating 4-tile/2-tile so the two
            # score PSUM tiles (4-bank + 2-bank) double-buffer within the
            # 8-bank budget while cutting per-activation overhead.
            GROUPS = ((0, 4), (4, 2), (6, 4), (10, 2), (12, 4))

            def attend(h, qb):
                q0 = qb * 512
                ps_ctx = ps_cx_pool.tile([128, 512], F32, tag="ctx")
                for g0, gn in GROUPS:
                    if gn == 4:
                        ps_sc = ps_big.tile([128, 2048], F32, tag="scb")
                    else:
                        ps_sc = ps_small.tile([128, 1024], F32, tag="scs")
                    for j in range(gn):
                        ktile = g0 + j
                        nc.tensor.matmul(
                            ps_sc[:, j * 512:(j + 1) * 512],
                            lhsT=sb_KT[:, h,
                                       ktile * 128:(ktile + 1) * 128],
                            rhs=sb_QT[:, h, q0:q0 + 512],
                            start=True, stop=True)
                    t_e = exps.tile([128, gn * 512], BF, tag=f"exp{gn}")
                    nc.scalar.activation(t_e, ps_sc, EXP, scale=0.125)
                    for j in range(gn):
                        ktile = g0 + j
                        nc.tensor.matmul(
                            ps_ctx[0:65, :],
                            lhsT=sb_V[:, ktile, h, :],
                            rhs=t_e[:, j * 512:(j + 1) * 512],
                            start=(ktile == 0), stop=(ktile == 15))
                # normalize: row 64 of ps_ctx is the denominator.
                # partition_broadcast only reads physical partition 0, so
                # DMA-move the row there first.
                t_rd = smalls.tile([128, 512], F32, tag="rd")
                nc.vector.tensor_copy(out=t_rd[64:65, :],
                                      in_=ps_ctx[64:65, :])
                nc.sync.dma_start(t_rd[0:1, :], t_rd[64:65, :])
                t_rc = smalls.tile([1, 512], F32, tag="rc")
                nc.vector.reciprocal_approx_fast(t_rc, t_rd[0:1, :])
                t_rdb = smalls.tile([64, 512], F32, tag="rdb")
                nc.gpsimd.partition_broadcast(t_rdb, t_rc)
                p = h // 2
                if h % 2 == 0:
                    nc.vector.tensor_mul(
                        out=sb_ctxT[0:64, p, q0:q0 + 512],
                        in0=ps_ctx[0:64, :], in1=t_rdb)
                else:
                    # stage at partitions 0:64, DMA to 64:128 of the pair
                    # slot (keeps DVE in/out partition offsets matched).
                    t_st = stage_pool.tile([64, 512], BF, tag="st")
                    nc.vector.tensor_mul(out=t_st, in0=ps_ctx[0:64, :],
                                         in1=t_rdb)
                    nc.sync.dma_start(sb_ctxT[64:128, p, q0:q0 + 512], t_st)

            for qb in range(4):
                for h in range(HPC):
                    attend(h, qb)
                    if h < NPAIR:
                        if qb < 3:
                            qproj_group(qb + 1, h)
                    elif qb > 0:
                        outproj_group((qb - 1) * 4 + (h - NPAIR))
            for st4 in range(4):
                outproj_group(12 + st4)


def _prep_core_inputs(query, key, value, Wq, Wk, Wv, Wo, bq, bk, bv, aug):
    """Build the 8 per-core input maps (host-side shard + transpose + cast)."""
    in_maps = []
    if aug:
        aug_blk = np.zeros((128, S), np.float32)
        aug_blk[0, :] = 1.0
    for b in range(B):
        qTb = query[b].T
        kTb = key[b].T
        vTb = value[b].T
        if aug:
            qTb = np.concatenate([qTb, aug_blk], axis=0)
            kTb = np.concatenate([kTb, aug_blk], axis=0)
            vTb = np.concatenate([vTb, aug_blk], axis=0)
        qTb = np.ascontiguousarray(qTb).astype(BF16)
        kTb = np.ascontiguousarray(kTb).astype(BF16)
        vTb = np.ascontiguousarray(vTb).astype(BF16)
        for g in range(2):
            rows = slice(g * DHG, (g + 1) * DHG)
            wq_t = Wq[rows, :].T
            wk_t = Wk[rows, :].T
            wv_t = Wv[rows, :].T
            if aug:
                wq_t = np.concatenate(
                    [wq_t, np.concatenate([bq[None, rows],
                                           np.zeros((127, DHG), np.float32)])])
                wk_t = np.concatenate(
                    [wk_t, np.concatenate([bk[None, rows],
                                           np.zeros((127, DHG), np.float32)])])
                wv_t = np.concatenate(
                    [wv_t, np.concatenate([bv[None, rows],
                                           np.zeros((127, DHG), np.float32)])])
            # woTp[p, pr, n] = Wo[n, g*512 + pr*128 + p] (head-pair packed)
            wo_g = Wo[:, g * DHG:(g + 1) * DHG]          # [1024, 512]
            woTp = np.ascontiguousarray(
                wo_g.T.reshape(NPAIR, 128, D_MODEL).transpose(1, 0, 2))
            in_maps.append({
                "qT": qTb,
                "kT": kTb,
                "vT": vTb,
                "wqT": np.ascontiguousarray(wq_t).astype(BF16),
                "wkT": np.ascontiguousarray(wk_t).astype(BF16),
                "wvT": np.ascontiguousarray(wv_t).astype(BF16),
                "woTp": woTp.astype(BF16),
            })
    return in_maps


def kernel(**inputs):
    global LAST_RESULT
    query = np.asarray(inputs["query"], np.float32)
    key = np.asarray(inputs["key"], np.float32)
    value = np.asarray(inputs["value"], np.float32)
    Wq = np.asarray(inputs["Wq"], np.float32)
    Wk = np.asarray(inputs["Wk"], np.float32)
    Wv = np.asarray(inputs["Wv"], np.float32)
    Wo = np.asarray(inputs["Wo"], np.float32)
    bq = np.asarray(inputs["bq"], np.float32)
    bk = np.asarray(inputs["bk"], np.float32)
    bv = np.asarray(inputs["bv"], np.float32)
    bo = np.asarray(inputs["bo"], np.float32)

    aug = bool(np.any(bq) or np.any(bk) or np.any(bv))
    kt = 9 if aug else 8
    nc = _PROGRAM_CACHE.get(kt)
    if nc is None:
        nc = _build_program(kt)
        _PROGRAM_CACHE[kt] = nc

    in_maps = _prep_core_inputs(query, key, value, Wq, Wk, Wv, Wo,
                                bq, bk, bv, aug)
    res = run_bass_kernel_spmd(
        nc, in_maps, core_ids=list(range(8)),
        trace=TRACE,
        **({"trace_cores": TRACE_CORES} if TRACE_CORES else {}))
    LAST_RESULT = res

    out = np.empty((B, S, D_MODEL), np.float32)
    for b in range(B):
        out[b] = res.results[2 * b]["out"] + res.results[2 * b + 1]["out"] + bo
    return out


# revision 11
# speedup vs baseline: 1.3251x; 1.3251x over previous
"""MultiHeadAttention Trainium2 kernel (B=4, S=2048, D=1024, H=16, Dh=64).

Sharding: 8 cores = 4 batches x 2 head-groups (8 heads each).  Each core
computes QKV projections for its (batch, head-group), full attention for its
8 heads, and a partial output projection (row-parallel over Wo).  The host
sums the two per-batch partials and adds the output bias.

Schedule (all matmuls bf16 inputs, fp32 PSUM accumulation):
  - K-proj and V-proj run first (streamed kT/vT over two DMA queues), in
    4-bank PSUM groups so drains overlap the next group's matmuls.
  - Attention runs qb-outer (4 query blocks x 8 heads).  The exp stream on
    ScalarE is the window bottleneck (~277us); Q-proj for qb+1 and the
    output projection for qb-1 slot into the PE slack between heads.
  - ctxT is stored head-PAIR packed [128, 4, S] (head 2p on partitions 0:64,
    head 2p+1 on 64:128) so out-proj matmuls contract a full K=128 of real
    data (no zero padding).  Odd heads' normalized ctx is staged through a
    [64,512] tile and DMA-moved to partitions 64:128.
  - scores matmuls keep the zero-padded K=128 contraction: half-array (K=64)
    matmuls never satisfy the PE activity monitor, pinning the clock at
    1.2 GHz.  Padding is free (matmul time is set by the moving free dim).
  - V is stored with a ones-column per head (V_ext [s, h, 65]) so the
    attention-value matmul also produces the softmax denominator (row 64 of
    ps_ctx).  Denominator -> reciprocal_approx_fast -> partition broadcast
    -> multiply.
"""

import sys

for _p in ("/opt/trn_rl_repo", "/root/.axon_site/_ro/trn_rl_repo"):
    if _p not in sys.path:
        sys.path.append(_p)

import numpy as np
import ml_dtypes

import concourse.bass as bass
import concourse.tile as tile
from concourse import bacc, mybir
from concourse.bass_utils import run_bass_kernel_spmd

BF16 = ml_dtypes.bfloat16
F32 = mybir.dt.float32
BF = mybir.dt.bfloat16

D_MODEL = 1024
NUM_HEADS = 16
HEAD_DIM = 64
B, S = 4, 2048
HPC = 8          # heads per core
DHG = HPC * HEAD_DIM  # 512, head dims per core
NPAIR = HPC // 2      # 4 head pairs per core

# knobs read by test.py
TRACE = False
TRACE_CORES = None
LAST_RESULT = None

_PROGRAM_CACHE = {}


def _build_program(kt: int) -> bass.Bass:
    """Build the per-core SPMD program. kt = contraction tiles over d_model
    (8 normally, 9 when inputs are augmented with a bias row)."""
    nc = bacc.Bacc("TRN2", debug=False, target_bir_lowering=False)

    kd = kt * 128
    qT = nc.dram_tensor("qT", [kd, S], BF, kind="ExternalInput").ap()
    kT = nc.dram_tensor("kT", [kd, S], BF, kind="ExternalInput").ap()
    vT = nc.dram_tensor("vT", [kd, S], BF, kind="ExternalInput").ap()
    wqT = nc.dram_tensor("wqT", [kd, DHG], BF, kind="ExternalInput").ap()
    wkT = nc.dram_tensor("wkT", [kd, DHG], BF, kind="ExternalInput").ap()
    wvT = nc.dram_tensor("wvT", [kd, DHG], BF, kind="ExternalInput").ap()
    woTp = nc.dram_tensor("woTp", [128, NPAIR, D_MODEL], BF,
                          kind="ExternalInput").ap()
    out = nc.dram_tensor("out", [S, D_MODEL], F32, kind="ExternalOutput").ap()

    with tile.TileContext(nc) as tc:
        _body(tc, qT, kT, vT, wqT, wkT, wvT, woTp, out, kt)
    nc.compile()
    return nc


def _body(tc, qT, kT, vT, wqT, wkT, wvT, woTp, out, kt):
    nc = tc.nc
    EXP = mybir.ActivationFunctionType.Exp

    with (
        tc.tile_pool(name="consts", bufs=1) as consts,
        tc.tile_pool(name="big", bufs=1) as big,
    ):
        sb_wq = consts.tile([128, kt, DHG], BF)
        sb_wo = consts.tile([128, NPAIR, D_MODEL], BF)
        sb_qres = consts.tile([128, kt, S], BF)
        sb_QT = big.tile([128, HPC, S], BF)
        sb_KT = big.tile([128, HPC, S], BF)
        sb_V = big.tile([128, 16, HPC, 65], BF)  # [s%128, s//128, h, :]
        sb_ctxT = big.tile([128, NPAIR, S], BF)

        # Scores lhsT/rhs dead partitions must be ZERO (not garbage: random
        # SBUF bits can decode to NaN and 0*NaN = NaN).  KT first: its zeros
        # gate the K-proj copies at ~10us.
        nc.vector.memset(sb_KT, 0.0)
        nc.vector.memset(sb_V[:, :, :, 64:65], 1.0)
        nc.vector.memset(sb_QT, 0.0)

        # ---- loads: sync queue carries wk + kT + wq + qres; the scalar
        # (ACT) HWDGE queue carries wv + vT + wo in parallel.  ACT is idle
        # during the projection phase so this doubles early DMA throughput.
        with tc.tile_pool(name="wkv", bufs=1) as wkv:
            sb_wk = wkv.tile([128, kt, DHG], BF)
            sb_wv = wkv.tile([128, kt, DHG], BF)
            nc.sync.dma_start(sb_wk, wkT.rearrange("(t p) m -> p t m", p=128))
            nc.scalar.dma_start(sb_wv,
                                wvT.rearrange("(t p) m -> p t m", p=128))

            with tc.tile_pool(name="ld", bufs=2 * kt) as loads:
                kin = []
                for nbp in range(2):
                    for ki in range(kt):
                        t = loads.tile([128, 1024], BF, tag="ld")
                        nc.sync.dma_start(
                            t, kT[ki * 128:(ki + 1) * 128,
                                  nbp * 1024:(nbp + 1) * 1024])
                        kin.append(t)
                nc.sync.dma_start(sb_wq,
                                  wqT.rearrange("(t p) m -> p t m", p=128))
                nc.sync.dma_start(sb_qres,
                                  qT.rearrange("(t p) f -> p t f", p=128))
                # vT tiles reuse the kT slots (pool round-robin); each load
                # waits for the K-proj reads of its slot to finish.
                vin = []
                for nbp in range(2):
                    for ki in range(kt):
                        t = loads.tile([128, 1024], BF, tag="ld")
                        nc.scalar.dma_start(
                            t, vT[ki * 128:(ki + 1) * 128,
                                  nbp * 1024:(nbp + 1) * 1024])
                        vin.append(t)
                nc.scalar.dma_start(sb_wo, woTp)

                # ---- K and V projections: 4-bank PSUM groups so each
                # group's drain copies overlap the next group's matmuls.
                with tc.tile_pool(name="ps_kv", bufs=8,
                                  space="PSUM") as psums:
                    for nbp in range(2):
                        for sg in range(2):     # s sub-blocks of 512
                            ps = [psums.tile([128, 512], F32, tag="pkv",
                                             name=f"pk{nbp}_{sg}_{i}")
                                  for i in range(NPAIR)]
                            for ki in range(kt):
                                t_in = kin[nbp * kt + ki]
                                for p in range(NPAIR):
                                    nc.tensor.matmul(
                                        ps[p],
                                        lhsT=sb_wk[:, ki,
                                                   p * 128:(p + 1) * 128],
                                        rhs=t_in[:, sg * 512:(sg + 1) * 512],
                                        start=(ki == 0), stop=(ki == kt - 1))
                            c0 = nbp * 1024 + sg * 512
                            for p in range(NPAIR):
                                for par in range(2):
                                    h = 2 * p + par
                                    sl = slice(par * 64, par * 64 + 64)
                                    nc.vector.tensor_copy(
                                        out=sb_KT[sl, h, c0:c0 + 512],
                                        in_=ps[p][sl, :])
                    # Q-proj for qb0 here: its matmuls become ready as soon
                    # as wq+qres land (~30us) and fill the PE stall while the
                    # first vT tiles wait on kT slot reuse.
                    for p in range(NPAIR):
                        psq = psums.tile([128, 512], F32, tag="pkv",
                                         name=f"pq0_{p}")
                        for ki in range(kt):
                            nc.tensor.matmul(
                                psq,
                                lhsT=sb_wq[:, ki, p * 128:(p + 1) * 128],
                                rhs=sb_qres[:, ki, 0:512],
                                start=(ki == 0), stop=(ki == kt - 1))
                        for par in range(2):
                            h = 2 * p + par
                            sl = slice(par * 64, par * 64 + 64)
                            nc.vector.tensor_copy(out=sb_QT[sl, h, 0:512],
                                                  in_=psq[sl, :])
                    # V: out[s-tile 128, dh 512]; lhsT = input tile.
                    for nbp in range(2):
                        for sg in range(2):     # st sub-groups of 4
                            ps = [psums.tile([128, 512], F32, tag="pkv",
                                             name=f"pv{nbp}_{sg}_{i}")
                                  for i in range(4)]
                            for ki in range(kt):
                                t_in = vin[nbp * kt + ki]
                                for i in range(4):
                                    sti = sg * 4 + i
                                    nc.tensor.matmul(
                                        ps[i],
                                        lhsT=t_in[:,
                                                  sti * 128:(sti + 1) * 128],
                                        rhs=sb_wv[:, ki, :],
                                        start=(ki == 0), stop=(ki == kt - 1))
                            for i in range(4):
                                st = nbp * 8 + sg * 4 + i
                                nc.vector.tensor_copy(
                                    out=sb_V[:, st, :, 0:64],
                                    in_=ps[i].rearrange("p (h d) -> p h d",
                                                        h=HPC))

        # ---- attention, qb-outer, with Q-proj/out-proj as PE filler -------
        with (
            tc.tile_pool(name="exps", bufs=4) as exps,
            tc.tile_pool(name="smalls", bufs=3) as smalls,
            tc.tile_pool(name="stage", bufs=2) as stage_pool,
            tc.tile_pool(name="outst", bufs=2) as outs_pool,
            tc.tile_pool(name="ps_sc", bufs=2, space="PSUM") as ps_sc_pool,
            tc.tile_pool(name="ps_cx", bufs=2, space="PSUM") as ps_cx_pool,
            tc.tile_pool(name="ps_fl", bufs=2, space="PSUM") as ps_fl_pool,
        ):
            def qproj_group(qb, p):
                q0 = qb * 512
                psq = ps_fl_pool.tile([128, 512], F32, tag="fl")
                for ki in range(kt):
                    nc.tensor.matmul(
                        psq,
                        lhsT=sb_wq[:, ki, p * 128:(p + 1) * 128],
                        rhs=sb_qres[:, ki, q0:q0 + 512],
                        start=(ki == 0), stop=(ki == kt - 1))
                for par in range(2):
                    h = 2 * p + par
                    sl = slice(par * 64, par * 64 + 64)
                    nc.vector.tensor_copy(out=sb_QT[sl, h, q0:q0 + 512],
                                          in_=psq[sl, :])

            def outproj_group(st):
                t_out = outs_pool.tile([128, D_MODEL], F32, tag="o")
                for nb2 in range(2):
                    ps_o = ps_fl_pool.tile([128, 512], F32, tag="fl")
                    for p in range(NPAIR):
                        nc.tensor.matmul(
                            ps_o,
                            lhsT=sb_ctxT[:, p, st * 128:(st + 1) * 128],
                            rhs=sb_wo[:, p, nb2 * 512:(nb2 + 1) * 512],
                            start=(p == 0), stop=(p == NPAIR - 1))
                    nc.vector.tensor_copy(
                        out=t_out[:, nb2 * 512:(nb2 + 1) * 512], in_=ps_o)
                nc.sync.dma_start(out[st * 128:(st + 1) * 128, :], t_out)

            def attend(h, qb):
                q0 = qb * 512
                ps_ctx = ps_cx_pool.tile([128, 512], F32, tag="ctx")
                for kp in range(8):  # pairs of key tiles
                    ps_sc = ps_sc_pool.tile([128, 1024], F32, tag="sc")
                    for j in range(2):
                        ktile = kp * 2 + j
                        nc.tensor.matmul(
                            ps_sc[:, j * 512:(j + 1) * 512],
                            lhsT=sb_KT[:, h,
                                       ktile * 128:(ktile + 1) * 128],
                            rhs=sb_QT[:, h, q0:q0 + 512],
                            start=True, stop=True)
                    t_e = exps.tile([128, 1024], BF, tag="exp")
                    nc.scalar.activation(t_e, ps_sc, EXP, scale=0.125)
                    for j in range(2):
                        ktile = kp * 2 + j
                        nc.tensor.matmul(
                            ps_ctx[0:65, :],
                            lhsT=sb_V[:, ktile, h, :],
                            rhs=t_e[:, j * 512:(j + 1) * 512],
                            start=(ktile == 0), stop=(ktile == 15))
                # normalize: row 64 of ps_ctx is the denominator.
                # partition_broadcast only reads physical partition 0, so
                # DMA-move the row there first.
                t_rd = smalls.tile([128, 512], F32, tag="rd")
                nc.vector.tensor_copy(out=t_rd[64:65, :],
                                      in_=ps_ctx[64:65, :])
                nc.sync.dma_start(t_rd[0:1, :], t_rd[64:65, :])
                t_rc = smalls.tile([1, 512], F32, tag="rc")
                nc.vector.reciprocal_approx_fast(t_rc, t_rd[0:1, :])
                t_rdb = smalls.tile([64, 512], F32, tag="rdb")
                nc.gpsimd.partition_broadcast(t_rdb, t_rc)
                p = h // 2
                if h % 2 == 0:
                    nc.vector.tensor_mul(
                        out=sb_ctxT[0:64, p, q0:q0 + 512],
                        in0=ps_ctx[0:64, :], in1=t_rdb)
                else:
                    # stage at partitions 0:64, DMA to 64:128 of the pair
                    # slot (keeps DVE in/out partition offsets matched).
                    t_st = stage_pool.tile([64, 512], BF, tag="st")
                    nc.vector.tensor_mul(out=t_st, in0=ps_ctx[0:64, :],
                                         in1=t_rdb)
                    nc.sync.dma_start(sb_ctxT[64:128, p, q0:q0 + 512], t_st)

            for qb in range(4):
                for h in range(HPC):
                    attend(h, qb)
                    if h < NPAIR:
                        if qb < 3:
                            qproj_group(qb + 1, h)
                    elif qb > 0:
                        outproj_group((qb - 1) * 4 + (h - NPAIR))
            for st4 in range(4):
                outproj_group(12 + st4)


def _prep_core_inputs(query, key, value, Wq, Wk, Wv, Wo, bq, bk, bv, aug):
    """Build the 8 per-core input maps (host-side shard + transpose + cast)."""
    in_maps = []
    if aug:
        aug_blk = np.zeros((128, S), np.float32)
        aug_blk[0, :] = 1.0
    for b in range(B):
        qTb = query[b].T
        kTb = key[b].T
        vTb = value[b].T
        if aug:
            qTb = np.concatenate([qTb, aug_blk], axis=0)
            kTb = np.concatenate([kTb, aug_blk], axis=0)
            vTb = np.concatenate([vTb, aug_blk], axis=0)
        qTb = np.ascontiguousarray(qTb).astype(BF16)
        kTb = np.ascontiguousarray(kTb).astype(BF16)
        vTb = np.ascontiguousarray(vTb).astype(BF16)
        for g in range(2):
            rows = slice(g * DHG, (g + 1) * DHG)
            wq_t = Wq[rows, :].T
            wk_t = Wk[rows, :].T
            wv_t = Wv[rows, :].T
            if aug:
                wq_t = np.concatenate(
                    [wq_t, np.concatenate([bq[None, rows],
                                           np.zeros((127, DHG), np.float32)])])
                wk_t = np.concatenate(
                    [wk_t, np.concatenate([bk[None, rows],
                                           np.zeros((127, DHG), np.float32)])])
                wv_t = np.concatenate(
                    [wv_t, np.concatenate([bv[None, rows],
                                           np.zeros((127, DHG), np.float32)])])
            # woTp[p, pr, n] = Wo[n, g*512 + pr*128 + p] (head-pair packed)
            wo_g = Wo[:, g * DHG:(g + 1) * DHG]          # [1024, 512]
            woTp = np.ascontiguousarray(
                wo_g.T.reshape(NPAIR, 128, D_MODEL).transpose(1, 0, 2))
            in_maps.append({
                "qT": qTb,
                "kT": kTb,
                "vT": vTb,
                "wqT": np.ascontiguousarray(wq_t).astype(BF16),
                "wkT": np.ascontiguousarray(wk_t).astype(BF16),
                "wvT": np.ascontiguousarray(wv_t).astype(BF16),
                "woTp": woTp.astype(BF16),
            })
    return in_maps


def kernel(**inputs):
    global LAST_RESULT
    query = np.asarray(inputs["query"], np.float32)
    key = np.asarray(inputs["key"], np.float32)
    value = np.asarray(inputs["value"], np.float32)
    Wq = np.asarray(inputs["Wq"], np.float32)
    Wk = np.asarray(inputs["Wk"], np.float32)
    Wv = np.asarray(inputs["Wv"], np.float32)
    Wo = np.asarray(inputs["Wo"], np.float32)
    bq = np.asarray(inputs["bq"], np.float32)
    bk = np.asarray(inputs["bk"], np.float32)
    bv = np.asarray(inputs["bv"], np.float32)
    bo = np.asarray(inputs["bo"], np.float32)

    aug = bool(np.any(bq) or np.any(bk) or np.any(bv))
    kt = 9 if aug else 8
    nc = _PROGRAM_CACHE.get(kt)
    if nc is None:
        nc = _build_program(kt)
        _PROGRAM_CACHE[kt] = nc

    in_maps = _prep_core_inputs(query, key, value, Wq, Wk, Wv, Wo,
                                bq, bk, bv, aug)
    res = run_bass_kernel_spmd(
        nc, in_maps, core_ids=list(range(8)),
        trace=TRACE,
        **({"trace_cores": TRACE_CORES} if TRACE_CORES else {}))
    LAST_RESULT = res

    out = np.empty((B, S, D_MODEL), np.float32)
    for b in range(B):
        out[b] = res.results[2 * b]["out"] + res.results[2 * b + 1]["out"] + bo
    return out
